# revision 11
# baseline (speedup 1.0000x reference)
"""ColumnSelfAttention Trainium2 Bass kernel.

Shards the num_cols axis (pure batch dim) across 8 NeuronCores. Each core:
  - projects its x shard to Q^T/K^T (feature-major) and V (token-major)
  - per column: scores = Q K^T per head (fp32r matmuls), softmax via ACT Exp
    with fused row-sum (accum_out) and per-column padding-mask fold,
    PE-transpose of probs for the PV matmul, output projection.
Host side pre-transposes x / weights, slices per core, and regathers.
"""

import sys
import types

import numpy as np

import concourse.bass as bass
import concourse.mybir as mybir
import concourse.tile as tile
from concourse.bass import ts
from concourse.bass_utils import run_bass_kernel_spmd

# bass_utils imports antenv.axon_hooks when tracing is requested; the agent
# image ships an antenv stub without that submodule. Provide a graceful
# default (hook=None -> tracing skipped) that a harness can upgrade.
try:
    from antenv import axon_hooks as _axon_hooks  # noqa: F401
except ImportError:
    _m = types.ModuleType("antenv.axon_hooks")
    _m._hook = None
    _m.get_axon_ntff_profile_hook = lambda: _m._hook
    _m.set_axon_ntff_profile_hook = lambda h: setattr(_m, "_hook", h)
    sys.modules["antenv.axon_hooks"] = _m
    import antenv
    antenv.axon_hooks = _m

# Problem constants (hardcoded per contract).
R = 128          # rows (attention sequence axis)
C = 256          # columns (batch axis for this attention)
E = 768          # embed dim
H = 12           # heads
D = 64           # head dim
SCALING = D ** -0.5
N_CORES = 8
C_SHARD = C // N_CORES          # 32 columns per core
T_SHARD = C_SHARD * R           # 4096 tokens per core
CHUNK_COLS = 2                  # columns per projection chunk (256 tokens)
KT_E = E // 128                 # 6 contraction tiles of 128

F32 = mybir.dt.float32
F32R = mybir.dt.float32r
AF = mybir.ActivationFunctionType

# Set by test harness to capture a profile.
_TRACE = False
_LAST_RESULTS = None

# walrus CTRL/S3_LW codegen rejects instructions with more than one
# embedded sync wait in this toolchain; spill excess waits to standalone
# event-semaphore waits on the same engine right before the instruction.
_MAX_WAITS_PER_INST = 1


def _legalize_waits(nc):
    for f in nc.m.functions:
        for bb in f.blocks:
            out = []
            for inst in bb.instructions:
                si = inst.sync_info
                if si is not None and si.on_wait and len(si.on_wait) > _MAX_WAITS_PER_INST:
                    waits = list(si.on_wait)
                    keep = waits[-_MAX_WAITS_PER_INST:]
                    for w in waits[:-_MAX_WAITS_PER_INST]:
                        wi = mybir.InstEventSemaphore(
                            name=nc.get_next_instruction_name(), engine=inst.engine
                        )
                        wi.sync_info = mybir.SyncInfo(on_wait=[w], on_update=[])
                        nc.register_instruction(wi)
                        out.append(wi)
                    si.on_wait = keep
                out.append(inst)
            bb.instructions[:] = out


def build_program(n_cols=C_SHARD, _stop_after=None):
    assert n_cols % CHUNK_COLS == 0
    t_total = n_cols * R
    nc = bass.Bass()

    xT = nc.declare_dram_parameter("xT", [E, t_total], F32R, isOutput=False)
    w_dram = {}
    for name in ("wq_t", "wk_t", "wv_t", "wo_t"):
        w_dram[name] = nc.declare_dram_parameter(name, [E, E], F32R, isOutput=False)
    bq = nc.declare_dram_parameter("bq", [E], F32, isOutput=False)
    bk = nc.declare_dram_parameter("bk", [E], F32, isOutput=False)
    bvb = nc.declare_dram_parameter("bv_bcast", [128, E], F32, isOutput=False)
    bob = nc.declare_dram_parameter("bo_bcast", [128, E], F32, isOutput=False)
    keep = nc.declare_dram_parameter("keep", [128, n_cols], F32, isOutput=False)
    ident = nc.declare_dram_parameter("ident", [128, 128], F32, isOutput=False)
    # hsel[:, 0] = 1 for partitions 0-63 else 0; hsel[:, 1] = complement.
    hsel = nc.declare_dram_parameter("hsel", [128, 2], F32, isOutput=False)
    probs_out = nc.declare_dram_parameter("probs", [n_cols, R, H, R], F32, isOutput=True)
    y_out = nc.declare_dram_parameter("y", [n_cols, R, E], F32, isOutput=True)

    TCH = CHUNK_COLS * R  # tokens per chunk

    with tile.TileContext(nc) as tc:
        with (
            tc.tile_pool(name="consts", bufs=1) as consts,
            tc.tile_pool(name="wpool", bufs=1) as wpool,
            tc.tile_pool(name="xpool", bufs=2) as xpool,
            tc.tile_pool(name="qkpool", bufs=2) as qkpool,
            tc.tile_pool(name="vpool", bufs=3) as vpool,
            tc.tile_pool(name="attnpool", bufs=2) as attnpool,
            tc.tile_pool(name="smallpool", bufs=4) as smallpool,
            tc.tile_pool(name="opool", bufs=2) as opool,
            tc.tile_pool(name="ps_proj", bufs=3, space="PSUM") as ps_proj,
            tc.tile_pool(name="ps_attn", bufs=2, space="PSUM") as ps_attn,
            tc.tile_pool(name="ps_ctx", bufs=2, space="PSUM") as ps_ctx,
        ):
            # ---- constants / weights ----
            w_sb = {}
            for name in ("wq_t", "wk_t", "wv_t", "wo_t"):
                t = wpool.tile([128, KT_E, E], F32R, tag=name)
                nc.sync.dma_start(
                    out=t, in_=w_dram[name][:, :].rearrange("(o p) m -> p o m", p=128)
                )
                w_sb[name] = t
            bq_sb = consts.tile([128, KT_E], F32, tag="bq")
            nc.sync.dma_start(out=bq_sb, in_=bq[:].rearrange("(o p) -> p o", p=128))
            bk_sb = consts.tile([128, KT_E], F32, tag="bk")
            nc.sync.dma_start(out=bk_sb, in_=bk[:].rearrange("(o p) -> p o", p=128))
            bvb_sb = consts.tile([128, E], F32, tag="bvb")
            nc.sync.dma_start(out=bvb_sb, in_=bvb[:, :])
            bob_sb = consts.tile([128, E], F32, tag="bob")
            nc.sync.dma_start(out=bob_sb, in_=bob[:, :])
            keep_sb = consts.tile([128, n_cols], F32, tag="keep")
            nc.sync.dma_start(out=keep_sb, in_=keep[:, :])
            id_sb = consts.tile([128, 128], F32, tag="ident")
            nc.sync.dma_start(out=id_sb, in_=ident[:, :])
            hsel_sb = consts.tile([128, 2], F32, tag="hsel")
            nc.sync.dma_start(out=hsel_sb, in_=hsel[:, :])

            xT_r = xT[:, :].rearrange("(o p) t -> p o t", p=128)

            for chunk in range(n_cols // CHUNK_COLS):
                t0 = chunk * TCH
                x_sb = xpool.tile([128, KT_E, TCH], F32R, tag="x")
                nc.sync.dma_start(out=x_sb, in_=xT_r[:, :, t0 : t0 + TCH])

                # ---- Q^T / K^T projections (feature-major) ----
                QT = qkpool.tile([128, KT_E, TCH], F32R, tag="QT")
                KT = qkpool.tile([128, KT_E, TCH], F32R, tag="KT")
                for dst, wname, bias_sb in ((QT, "wq_t", bq_sb), (KT, "wk_t", bk_sb)):
                    for mt in range(KT_E):
                        ps = ps_proj.tile([128, 512], F32, tag="ps_proj", name="ps_qk")[:, :TCH]
                        for kt in range(KT_E):
                            nc.tensor.matmul(
                                ps,
                                w_sb[wname][:, kt, ts(mt, 128)],
                                x_sb[:, kt, :],
                                start=(kt == 0),
                                stop=(kt == KT_E - 1),
                            )
                        nc.scalar.activation(
                            out=dst[:, mt, :],
                            in_=ps,
                            func=AF.Identity,
                            bias=bias_sb[:, mt : mt + 1],
                            scale=1.0,
                        )

                if _stop_after == "proj_qk":
                    continue
                for cl in range(CHUNK_COLS):
                    c = chunk * CHUNK_COLS + cl
                    tok = slice(cl * R, (cl + 1) * R)

                    # ---- V projection for this column (token-major) ----
                    V = vpool.tile([128, E], F32R, tag="V")
                    for half in range(2):
                        ps = ps_proj.tile([128, 512], F32, tag="ps_proj", name="ps_v")[:, :384]
                        for kt in range(KT_E):
                            nc.tensor.matmul(
                                ps,
                                x_sb[:, kt, tok],
                                w_sb["wv_t"][:, kt, ts(half, 384)],
                                start=(kt == 0),
                                stop=(kt == KT_E - 1),
                            )
                        nc.vector.tensor_add(
                            out=V[:, ts(half, 384)], in0=ps, in1=bvb_sb[:, ts(half, 384)]
                        )

                    if _stop_after == "proj":
                        continue
                    # ---- scores + softmax ----
                    # Per-head zero-padded Q: consecutive matmuls must not
                    # switch operand base partitions (0<->64 transitions hang
                    # the PE), so every scores matmul contracts the full 128
                    # partitions with the other head's rows zeroed.
                    QTz = attnpool.tile([128, H, R], F32R, tag="QTz")
                    for h in range(H):
                        nc.vector.tensor_scalar_mul(
                            out=QTz[:, h, :],
                            in0=QT[:, h // 2, tok],
                            scalar1=hsel_sb[:, (h % 2) : (h % 2) + 1],
                        )
                    exp_sb = attnpool.tile([128, H, R], F32, tag="exp")
                    sums = smallpool.tile([128, H], F32, tag="sums")
                    recip = smallpool.tile([128, H], F32, tag="recip")
                    for g in range(3):  # 4 heads per PSUM bank
                        ps_s = ps_attn.tile([128, 512], F32, tag="ps_attn")
                        for hh in range(4):
                            h = g * 4 + hh
                            nc.tensor.matmul(
                                ps_s[:, ts(hh, R)],
                                QTz[:, h, :],
                                KT[:, h // 2, tok],
                                start=True,
                                stop=True,
                            )
                        for hh in range(4):
                            h = g * 4 + hh
                            nc.scalar.activation(
                                out=exp_sb[:, h, :],
                                in_=ps_s[:, ts(hh, R)],
                                func=AF.Exp,
                                scale=keep_sb[:, c : c + 1],
                                accum_out=sums[:, h : h + 1],
                            )
                    nc.vector.reciprocal(out=recip, in_=sums)
                    # normalize in place: probs = exp * (1/sum)
                    for h in range(H):
                        nc.vector.tensor_scalar_mul(
                            out=exp_sb[:, h, :],
                            in0=exp_sb[:, h, :],
                            scalar1=recip[:, h : h + 1],
                        )
                    nc.sync.dma_start(out=probs_out[c], in_=exp_sb)

                    if _stop_after == "softmax":
                        continue
                    # ---- probs^T via PE transpose ----
                    probsT = attnpool.tile([128, H, R], F32R, tag="probsT")
                    for g in range(3):
                        ps_t = ps_attn.tile([128, 512], F32, tag="ps_attn")
                        for hh in range(4):
                            h = g * 4 + hh
                            nc.tensor.transpose(ps_t[:, ts(hh, R)], exp_sb[:, h, :], id_sb)
                        nc.scalar.activation(
                            out=probsT[:, ts(g, 4), :],
                            in_=ps_t[:, :].rearrange("p (h j) -> p h j", h=4),
                            func=AF.Copy,
                        )

                    if _stop_after == "probsT":
                        continue
                    # ---- context (natural layout): ctx[i, (h,d)] = probs @ V ----
                    ctx_nat = opool.tile([128, H, D], F32, tag="ctxn")
                    for h0, nh in ((0, 8), (8, 4)):
                        ps_c = ps_ctx.tile([128, 512], F32, tag="ps_ctx", name="ps_cn")
                        for hh in range(nh):
                            h = h0 + hh
                            nc.tensor.matmul(
                                ps_c[:, ts(hh, D)],
                                probsT[:, h, :],
                                V[:, ts(h, D)],
                                start=True,
                                stop=True,
                            )
                        nc.vector.tensor_copy(
                            out=ctx_nat[:, h0 : h0 + nh, :],
                            in_=ps_c[:, : nh * D].rearrange("p (h d) -> p h d", h=nh),
                        )

                    if _stop_after == "ctx":
                        continue
                    # ---- ctx^T via PE transpose (feature-major for out-proj) ----
                    ctxT = opool.tile([128, KT_E, R], F32R, tag="ctxT")
                    for half in range(2):
                        ps_t2 = ps_attn.tile([128, 512], F32, tag="ps_attn", name="ps_ct")
                        for kt3 in range(3):
                            kt = half * 3 + kt3
                            nc.tensor.transpose(
                                ps_t2[:, ts(kt3, 128)],
                                ctx_nat[:, 2 * kt : 2 * kt + 2, :].rearrange(
                                    "p h d -> p (h d)"
                                ),
                                id_sb,
                            )
                        nc.scalar.activation(
                            out=ctxT[:, ts(half, 3), :],
                            in_=ps_t2[:, :384].rearrange("p (k i) -> p k i", k=3),
                            func=AF.Copy,
                        )

                    if _stop_after == "ctxT":
                        continue
                    # ---- output projection ----
                    y_sb = opool.tile([128, E], F32, tag="y")
                    for half in range(2):
                        ps_o = ps_ctx.tile([128, 512], F32, tag="ps_ctx", name="ps_o")[:, :384]
                        for kt in range(KT_E):
                            nc.tensor.matmul(
                                ps_o,
                                ctxT[:, kt, :],
                                w_sb["wo_t"][:, kt, ts(half, 384)],
                                start=(kt == 0),
                                stop=(kt == KT_E - 1),
                            )
                        nc.vector.tensor_add(
                            out=y_sb[:, ts(half, 384)],
                            in0=ps_o,
                            in1=bob_sb[:, ts(half, 384)],
                        )
                    nc.sync.dma_start(out=y_out[c], in_=y_sb)

    _legalize_waits(nc)
    return nc


def _host_inputs(x, self_attn_padding_mask, wq, bq, wk, bk, wv, bv, wo, bo, n_cores=N_CORES):
    """Slice + pre-transpose host-side inputs into per-core in_maps."""
    f32 = np.float32
    wq_t = np.ascontiguousarray((wq.astype(f32) * f32(SCALING)).T)
    wk_t = np.ascontiguousarray(wk.astype(f32).T)
    wv_t = np.ascontiguousarray(wv.astype(f32).T)
    wo_t = np.ascontiguousarray(wo.astype(f32).T)
    bq_s = np.ascontiguousarray(bq.astype(f32) * f32(SCALING))
    bk_c = np.ascontiguousarray(bk.astype(f32))
    bvb = np.ascontiguousarray(np.broadcast_to(bv.astype(f32), (128, E)))
    bob = np.ascontiguousarray(np.broadcast_to(bo.astype(f32), (128, E)))
    ident = np.eye(128, dtype=f32)
    hsel = np.zeros((128, 2), dtype=f32)
    hsel[:64, 0] = 1.0
    hsel[64:, 1] = 1.0
    keep_full = (~np.asarray(self_attn_padding_mask)[0]).astype(f32)  # [C]
    xT_full = np.ascontiguousarray(x.astype(f32)[:, :, 0, :].transpose(2, 1, 0))  # [E, C, R]

    in_maps = []
    for core in range(n_cores):
        cs = slice(core * C_SHARD, (core + 1) * C_SHARD)
        in_maps.append(
            {
                "xT": np.ascontiguousarray(xT_full[:, cs, :]).reshape(E, T_SHARD),
                "wq_t": wq_t,
                "wk_t": wk_t,
                "wv_t": wv_t,
                "wo_t": wo_t,
                "bq": bq_s,
                "bk": bk_c,
                "bv_bcast": bvb,
                "bo_bcast": bob,
                "keep": np.ascontiguousarray(
                    np.broadcast_to(keep_full[cs], (128, C_SHARD))
                ),
                "ident": ident,
                "hsel": hsel,
            }
        )
    return in_maps


def kernel(x, self_attn_padding_mask, wq, bq, wk, bk, wv, bv, wo, bo):
    global _LAST_RESULTS
    in_maps = _host_inputs(x, self_attn_padding_mask, wq, bq, wk, bk, wv, bv, wo, bo)
    nc = build_program()
    res = run_bass_kernel_spmd(nc, in_maps, core_ids=list(range(N_CORES)), trace=_TRACE)
    _LAST_RESULTS = res

    probs_parts = []
    y_parts = []
    for r in res.results:
        # probs per core: [C_SHARD, R(i), H, R(j)] -> [H, C_SHARD, R, R]
        probs_parts.append(np.ascontiguousarray(r["probs"].transpose(2, 0, 1, 3)))
        # y per core: [C_SHARD, R, E] -> [R, C_SHARD, E]
        y_parts.append(np.ascontiguousarray(r["y"].transpose(1, 0, 2)))
    attn_probs = np.concatenate(probs_parts, axis=1)[:, :, None, :, :]  # [H, C, 1, R, R]
    output = np.concatenate(y_parts, axis=1)[:, :, None, :]  # [R, C, 1, E]
    return output.astype(np.float32), attn_probs.astype(np.float32)
